# revision 1
# baseline (speedup 1.0000x reference)
"""GAT (graph attention) layer on 8 Trainium2 NeuronCores.

Strategy (dst-partitioned edge parallelism):
  - Nodes are split into 8 contiguous row-ranges (one per core).
  - Phase A (per core): project the core's node shard:
        [Wh | e_s | e_d] = h_shard @ [Wmat | A_s | A_d] + bias
    using TensorE in bf16 (fp32 PSUM accumulation), where
        A_s[f,h] = sum_o W[h,f,o] a_src[h,o]   (score vectors folded into the
        A_d[f,h] = sum_o W[h,f,o] a_dst[h,o]    projection weights on the host)
    The per-node gather table rows [Wh+Wb | e_s] (bf16) are AllGathered so
    every core holds the full table; e_d (+score bias consts) stays local
    because every edge's dst lives in the owning core's shard.
  - Phase B (per core): edges are grouped (on host) by destination into
    "windows" of <=127 dst nodes, each window holding <= C*128 edges.
    Per 128-edge chunk:
      * indirect-DMA gather of table rows by src        -> G [128, HO+H]
      * per-edge e_d gathered by dst (bulk indirect DMA)
      * w = exp(leakyrelu(e_s+e_d)) = max(exp(s), exp(0.2 s))  (ACT + DVE)
      * one-hot A[e,v] = (iota - dstl == 0)             (one DVE op per chunk)
      * numerator/denominator via a single PSUM-accumulated matmul:
            out[v, :] += A.T @ [w*Wh | w]
    Softmax normalization happens once per window on the aggregated sums
    (exactly equivalent to the reference's max-subtracted softmax, since the
    scores are bounded; isolated nodes produce 0 like the reference).
  - Host does only index/layout work: edge bucketing, packing, transposes,
    and final row/column unscrambles.
"""
import os
import sys

sys.path.insert(0, "/opt/trn_rl_repo")

import numpy as np
import ml_dtypes

import concourse.bass as bass
import concourse.bacc as bacc
import concourse.tile as tile
import concourse.mybir as mybir

BF16 = np.dtype(ml_dtypes.bfloat16)
P = 128

# Full-problem configuration (matches reference.setup_inputs()).
FULL_CFG = dict(
    N=50000, F=512, H=8, O=32, ALPHA=0.2, NCORES=8,
)

_LAST_RESULTS = {}  # exposed for test.py (exec time etc.)


# --------------------------------------------------------------------------
# Host-side planning
# --------------------------------------------------------------------------

def _plan(cfg, src, dst):
    """Partition nodes/edges across cores and pack edges into windows/chunks.

    Returns a dict of per-core device arrays + unscramble info.
    """
    import heapq

    N, NCORES = cfg["N"], cfg["NCORES"]
    NS = cfg["NS"]            # padded rows per core (multiple of 128)
    NW = cfg["NW"]            # windows per core
    E = src.shape[0]

    deg = np.bincount(dst, minlength=N).astype(np.int64)

    slot_of = np.empty(N, np.int32)   # window within core
    pos_of = np.empty(N, np.int32)    # position within window (0..126)

    max_win_edges = 0
    for c in range(NCORES):
        lo, hi = NS * c, min(NS * (c + 1), N)
        nodes = np.arange(lo, hi)
        order = nodes[np.argsort(-deg[lo:hi], kind="stable")]
        # greedy: heaviest node into least-loaded window with room
        heap = [(0, 0, w) for w in range(NW)]
        heapq.heapify(heap)
        for n in order:
            load, cnt, w = heapq.heappop(heap)
            slot_of[n] = w
            pos_of[n] = cnt
            cnt += 1
            load += deg[n]
            if cnt < P - 1:  # positions 0..126 only; 127 reserved for pads
                heapq.heappush(heap, (load, cnt, w))
            if load > max_win_edges:
                max_win_edges = load
    C = max(1, -(-max_win_edges // P))  # chunks per window
    cap = C * P

    # per-core packed edge arrays [P, NW*C]
    src_idx = np.zeros((NCORES, P, NW * C), np.int32)
    dst_idx = np.zeros((NCORES, P, NW * C), np.int32)
    dstl = np.full((NCORES, P, NW * C), 127.0, np.float32)

    core_of = dst // NS
    for c in range(NCORES):
        m = core_of == c
        s_c, d_c = src[m], dst[m]
        w_c = slot_of[d_c]
        order = np.argsort(w_c, kind="stable")
        s_c, d_c, w_c = s_c[order], d_c[order], w_c[order]
        counts = np.bincount(w_c, minlength=NW)
        assert counts.max() <= cap, (counts.max(), cap)
        # position of each edge within its window
        off = np.concatenate([[0], np.cumsum(counts)[:-1]])
        within = np.arange(len(s_c)) - off[w_c]
        j = w_c * cap + within            # flat slot-major index
        p = j % P
        k = j // P
        col = k  # = window*C + chunk
        src_arr = src_idx[c]
        dst_arr = dst_idx[c]
        dstl_arr = dstl[c]
        src_arr[p, col] = s_c
        dst_arr[p, col] = d_c - NS * c
        dstl_arr[p, col] = pos_of[d_c]

    return dict(
        C=C,
        src_idx=src_idx,
        dst_idx=dst_idx,
        dstl=dstl,
        slot_of=slot_of,
        pos_of=pos_of,
    )


def _host_weights(cfg, W, Wb, a, ab):
    """Build extended projection weights / bias (o-major, h-inner layout)."""
    H, F, O = W.shape
    a_src, a_dst = a[:, :O], a[:, O:]
    Wmat = W.transpose(1, 2, 0).reshape(F, O * H)          # [F, (o,h)]
    A_s = np.einsum("hfo,ho->fh", W, a_src)
    A_d = np.einsum("hfo,ho->fh", W, a_dst)
    Wext = np.concatenate([Wmat, A_s, A_d], axis=1)        # [F, OH+2H]
    c_s = (Wb * a_src).sum(1)
    c_d = (Wb * a_dst).sum(1)
    bext = np.concatenate([Wb.T.reshape(-1), np.zeros(H, np.float32),
                           c_s + c_d + ab]).astype(np.float32)
    return Wext.astype(np.float32), bext


# --------------------------------------------------------------------------
# Device program
# --------------------------------------------------------------------------

def build_gat_bass(cfg):
    """Build the SPMD Bass program. Returns nc."""
    N, F, H, O, NCORES = cfg["N"], cfg["F"], cfg["H"], cfg["O"], cfg["NCORES"]
    NS, NW, C = cfg["NS"], cfg["NW"], cfg["C"]
    HO = H * O
    TD = HO + H          # table row: Wh + e_s
    AD = HO + 2 * H      # phase-A psum width: Wh + e_s + e_d
    NT = NS // P         # phase-A node tiles per core
    KT = F // P          # contraction tiles
    NG = N               # padded global rows = NS * NCORES
    NSG = NS * NCORES

    bf = mybir.dt.bfloat16
    f32 = mybir.dt.float32

    nc = bacc.Bacc("TRN2", target_bir_lowering=False, debug=False,
                   num_devices=NCORES)

    hT = nc.dram_tensor("hT", [F, NS], bf, kind="ExternalInput")
    wext = nc.dram_tensor("wext", [F, AD], bf, kind="ExternalInput")
    bext = nc.dram_tensor("bext", [1, AD], bf, kind="ExternalInput")
    ones1 = nc.dram_tensor("ones1", [1, P], bf, kind="ExternalInput")
    iota = nc.dram_tensor("iota", [P, P], bf, kind="ExternalInput")
    src_idx = nc.dram_tensor("src_idx", [P, NW * C], mybir.dt.int32,
                             kind="ExternalInput")
    dst_idx = nc.dram_tensor("dst_idx", [P, NW * C], mybir.dt.int32,
                             kind="ExternalInput")
    dstl = nc.dram_tensor("dstl", [P, NW * C], f32, kind="ExternalInput")

    out_local = nc.dram_tensor("out_local", [NW * P, HO], f32,
                               kind="ExternalOutput")
    dbg = cfg.get("debug_outputs")
    if dbg:
        dbg_tbl = nc.dram_tensor("dbg_tbl", [NS * NCORES, TD // 2],
                                 mybir.dt.int32, kind="ExternalOutput")
        dbg_ed = nc.dram_tensor("dbg_ed", [P, NW * C * H], f32,
                                kind="ExternalOutput")
        dbg_g = nc.dram_tensor("dbg_g", [P, C * (TD // 2)], mybir.dt.int32,
                               kind="ExternalOutput")
        dbg_w = nc.dram_tensor("dbg_w", [P, C * H], f32, kind="ExternalOutput")
        dbg_gp = nc.dram_tensor("dbg_gp", [P, C * TD], mybir.dt.uint16,
                                kind="ExternalOutput")
        dbg_agg = nc.dram_tensor("dbg_agg", [P, TD], f32, kind="ExternalOutput")

    with tile.TileContext(nc) as tc:
        with (
            tc.tile_pool(name="dram", bufs=1, space="DRAM") as dram,
            tc.tile_pool(name="const", bufs=1) as cpool,
        ):
            # bf16 multi-index indirect gather is miscompiled on HW for
            # partitions >= 64, so the table is stored/gathered as int32
            # (same bytes, TD/2 columns) and bitcast back in SBUF.
            TDI = TD
            tbl_local = dram.tile([NS, TD], bf)
            tbl_global = dram.tile(
                [NSG, TD], bf,
                addr_space="Shared" if NCORES > 4 else "Local")
            ed_local = dram.tile([NS, H], f32)

            iota_t = cpool.tile([P, P], bf)
            nc.sync.dma_start(out=iota_t[:], in_=iota[:, :])
            ones_t = cpool.tile([1, P], bf)
            nc.sync.dma_start(out=ones_t[:], in_=ones1[:, :])
            bext_t = cpool.tile([1, AD], bf)
            nc.sync.dma_start(out=bext_t[:], in_=bext[:, :])
            srci_t = cpool.tile([P, NW * C], mybir.dt.int32)
            nc.sync.dma_start(out=srci_t[:], in_=src_idx[:, :])
            dsti_t = cpool.tile([P, NW * C], mybir.dt.int32)
            nc.sync.dma_start(out=dsti_t[:], in_=dst_idx[:, :])
            dstl_t = cpool.tile([P, NW * C], f32)
            nc.sync.dma_start(out=dstl_t[:], in_=dstl[:, :])

            # ---------------- Phase A: projection ----------------
            with (
                tc.tile_pool(name="pa_sb", bufs=1) as pa,
                tc.tile_pool(name="pa_ps", bufs=2, space="PSUM") as pa_ps,
                tc.tile_pool(name="pa_cp", bufs=2) as pa_cp,
            ):
                hT_t = pa.tile([P, KT * NS], bf, tag="hT")
                for kk in range(KT):
                    nc.sync.dma_start(out=hT_t[:, kk * NS:(kk + 1) * NS],
                                      in_=hT[kk * P:(kk + 1) * P, :])
                wext_t = pa.tile([P, KT * AD], bf, tag="wext")
                for kk in range(KT):
                    nc.sync.dma_start(out=wext_t[:, kk * AD:(kk + 1) * AD],
                                      in_=wext[kk * P:(kk + 1) * P, :])

                stage = pa.tile([P, NT * TD], bf, tag="stage")
                ed_stage = pa.tile([P, NT * H], f32, tag="ed_stage")

                for t in range(NT):
                    psA = pa_ps.tile([P, AD], f32, tag="psA")
                    for kk in range(KT):
                        nc.tensor.matmul(
                            out=psA[:],
                            lhsT=hT_t[:, kk * NS + t * P: kk * NS + (t + 1) * P],
                            rhs=wext_t[:, kk * AD:(kk + 1) * AD],
                            start=(kk == 0), stop=False)
                    nc.tensor.matmul(out=psA[:], lhsT=ones_t[:],
                                     rhs=bext_t[:], start=False, stop=True)
                    nc.vector.tensor_copy(
                        out=stage[:, t * TD:(t + 1) * TD], in_=psA[:, 0:TD])
                    nc.vector.tensor_copy(
                        out=ed_stage[:, t * H:(t + 1) * H],
                        in_=psA[:, TD:TD + H])

                nc.sync.dma_start(
                    out=tbl_local[:].rearrange("(t p) d -> p t d", p=P),
                    in_=stage[:].rearrange("p (t d) -> p t d", t=NT))
                nc.sync.dma_start(
                    out=ed_local[:].rearrange("(t p) d -> p t d", p=P),
                    in_=ed_stage[:].rearrange("p (t d) -> p t d", t=NT))

            if cfg.get("skip_collective"):
                nc.sync.dma_start(out=tbl_global[0:NS, :], in_=tbl_local[:])
            else:
                nc.gpsimd.collective_compute(
                    "AllGather",
                    mybir.AluOpType.bypass,
                    replica_groups=[list(range(NCORES))],
                    ins=[tbl_local.opt()],
                    outs=[tbl_global.opt()],
                )

            # ---------------- Phase B: edges ----------------
            with (
                tc.tile_pool(name="ed_sb", bufs=1) as edp,
                tc.tile_pool(name="g_sb", bufs=3) as gp,
                tc.tile_pool(name="a_sb", bufs=2) as apool,
                tc.tile_pool(name="w_sb", bufs=2) as wpool,
                tc.tile_pool(name="o_sb", bufs=2) as opool,
                tc.tile_pool(name="agg_ps", bufs=2, space="PSUM") as aggp,
            ):
                if dbg:
                    nc.sync.dma_start(out=dbg_tbl[:, :], in_=tbl_global[:])
                # per-edge e_d gather: HW indirect DMA honors exactly ONE
                # index per partition per call, so gather chunk-by-chunk.
                ed_big = edp.tile([P, NW * C * H], f32)
                for j in range(NW * C):
                    nc.gpsimd.indirect_dma_start(
                        out=ed_big[:, j * H:(j + 1) * H],
                        out_offset=None,
                        in_=ed_local[:],
                        in_offset=bass.IndirectOffsetOnAxis(
                            ap=dsti_t[:, j:j + 1], axis=0),
                    )

                if dbg:
                    nc.sync.dma_start(out=dbg_ed[:, :], in_=ed_big[:])
                for s in range(NW):
                    g_raw = gp.tile([P, C * TDI], bf, tag="g")
                    for k in range(C):
                        nc.gpsimd.indirect_dma_start(
                            out=g_raw[:, k * TDI:(k + 1) * TDI],
                            out_offset=None,
                            in_=tbl_global[:],
                            in_offset=bass.IndirectOffsetOnAxis(
                                ap=srci_t[:, s * C + k:s * C + k + 1], axis=0),
                        )
                    g_t = g_raw[:]

                    # scores s = e_s + e_d  [P, C, H] fp32
                    s_t = wpool.tile([P, C * H], f32, tag="s")
                    nc.vector.tensor_tensor(
                        out=s_t[:].rearrange("p (k x) -> p k x", k=C),
                        in0=g_t.rearrange("p (k d) -> p k d", k=C)[:, :, HO:TD],
                        in1=ed_big[:, s * C * H:(s + 1) * C * H].rearrange(
                            "p (k x) -> p k x", k=C),
                        op=mybir.AluOpType.add)

                    # w = max(exp(s), exp(0.2 s))
                    w1 = wpool.tile([P, C * H], f32, tag="w1")
                    nc.scalar.activation(out=w1[:], in_=s_t[:],
                                         func=mybir.ActivationFunctionType.Exp)
                    w2 = wpool.tile([P, C * H], f32, tag="w2")
                    nc.scalar.activation(out=w2[:], in_=s_t[:],
                                         func=mybir.ActivationFunctionType.Exp,
                                         scale=float(cfg["ALPHA"]))

                    # gp holds [w*Wh | w] per chunk, width TD
                    gp_t = gp.tile([P, C * TD], bf, tag="gp")
                    gp4 = gp_t[:].rearrange("p (k d) -> p k d", k=C)
                    # w -> gp[:, :, HO:TD] (bf16)
                    nc.vector.tensor_tensor(
                        out=gp4[:, :, HO:TD],
                        in0=w1[:].rearrange("p (k x) -> p k x", k=C),
                        in1=w2[:].rearrange("p (k x) -> p k x", k=C),
                        op=mybir.AluOpType.max)
                    # product: gp[:, :, (o h)] = G * w (broadcast over o)
                    nc.vector.tensor_tensor(
                        out=gp4[:, :, 0:HO].rearrange(
                            "p k (o x) -> p k o x", o=O),
                        in0=g_t.rearrange(
                            "p (k d) -> p k d", k=C)[:, :, 0:HO].rearrange(
                            "p k (o x) -> p k o x", o=O),
                        in1=gp4[:, :, HO:TD][:, :, None, :].to_broadcast(
                            [P, C, O, H]),
                        op=mybir.AluOpType.mult)

                    # one-hot per chunk + aggregation matmul
                    a_t = apool.tile([P, C * P], bf, tag="a")
                    for k in range(C):
                        nc.vector.tensor_scalar(
                            out=a_t[:, k * P:(k + 1) * P],
                            in0=iota_t[:],
                            scalar1=dstl_t[:, s * C + k: s * C + k + 1],
                            scalar2=0.0,
                            op0=mybir.AluOpType.subtract,
                            op1=mybir.AluOpType.is_equal)

                    agg = aggp.tile([P, TD], f32, tag="agg")
                    for k in range(C):
                        nc.tensor.matmul(
                            out=agg[:],
                            lhsT=a_t[:, k * P:(k + 1) * P],
                            rhs=gp_t[:, k * TD:(k + 1) * TD],
                            start=(k == 0), stop=(k == C - 1))

                    if dbg and s == 0:
                        nc.sync.dma_start(out=dbg_g[:, :], in_=g_raw[:])
                        nc.sync.dma_start(
                            out=dbg_gp[:, :],
                            in_=gp_t[:].bitcast(mybir.dt.uint16))
                        agg_sbuf_dbg = opool.tile([P, TD], f32, tag="dbgagg")
                        nc.vector.tensor_copy(out=agg_sbuf_dbg[:], in_=agg[:])
                        nc.sync.dma_start(out=dbg_agg[:, :],
                                          in_=agg_sbuf_dbg[:])
                        w_dbg = wpool.tile([P, C * H], f32, tag="wdbg")
                        nc.vector.tensor_tensor(
                            out=w_dbg[:], in0=w1[:], in1=w2[:],
                            op=mybir.AluOpType.max)
                        nc.sync.dma_start(out=dbg_w[:, :], in_=w_dbg[:])
                    # normalize: out = num / max(denom, tiny)
                    den = opool.tile([P, H], f32, tag="den")
                    nc.vector.tensor_scalar(
                        out=den[:], in0=agg[:, HO:TD],
                        scalar1=1e-30, scalar2=None,
                        op0=mybir.AluOpType.max)
                    rec = opool.tile([P, H], f32, tag="rec")
                    nc.vector.reciprocal(out=rec[:], in_=den[:])
                    o_t = opool.tile([P, HO], f32, tag="o")
                    nc.vector.tensor_tensor(
                        out=o_t[:].rearrange("p (o x) -> p o x", o=O),
                        in0=agg[:, 0:HO].rearrange("p (o x) -> p o x", o=O),
                        in1=rec[:][:, None, :].to_broadcast([P, O, H]),
                        op=mybir.AluOpType.mult)
                    nc.sync.dma_start(
                        out=out_local[s * P:(s + 1) * P, :], in_=o_t[:])

    return nc


# --------------------------------------------------------------------------
# Execution (mirrors bass2jax.run_bass_via_pjrt, adding steady-state timing)
# --------------------------------------------------------------------------

def _run_pjrt_timed(nc, in_maps, n_cores, n_reps=1):
    import time

    import jax
    from jax.sharding import Mesh, PartitionSpec
    from jax.experimental.shard_map import shard_map

    from concourse import bass2jax
    from concourse import mybir as mb

    bass2jax.install_neuronx_cc_hook()

    partition_name = (nc.partition_id_tensor.name
                      if nc.partition_id_tensor else None)

    in_names, out_names, out_avals, zero_outs = [], [], [], []
    for alloc in nc.m.functions[0].allocations:
        if not isinstance(alloc, mb.MemoryLocationSet):
            continue
        name = alloc.memorylocations[0].name
        if alloc.kind == "ExternalInput":
            if name != partition_name:
                in_names.append(name)
        elif alloc.kind == "ExternalOutput":
            shape = tuple(alloc.tensor_shape)
            dtype = mb.dt.np(alloc.dtype)
            out_names.append(name)
            out_avals.append(jax.core.ShapedArray(shape, dtype))
            zero_outs.append(np.zeros(shape, dtype))
    n_params = len(in_names)
    n_outs = len(out_avals)
    all_in_names = list(in_names) + out_names
    if partition_name is not None:
        all_in_names.append(partition_name)
    donate = tuple(range(n_params, n_params + n_outs))

    def _body(*args):
        operands = list(args)
        if partition_name is not None:
            operands.append(bass2jax.partition_id_tensor())
        outs = bass2jax._bass_exec_p.bind(
            *operands,
            out_avals=tuple(out_avals),
            in_names=tuple(all_in_names),
            out_names=tuple(out_names),
            lowering_input_output_aliases=(),
            sim_require_finite=True,
            sim_require_nnan=True,
            nc=nc,
        )
        return tuple(outs)

    devices = jax.devices()[:n_cores]
    mesh = Mesh(np.asarray(devices), ("core",))
    in_specs = (PartitionSpec("core"),) * (n_params + n_outs)
    out_specs = (PartitionSpec("core"),) * len(out_names)
    sharded = jax.jit(
        shard_map(_body, mesh=mesh, in_specs=in_specs, out_specs=out_specs,
                  check_rep=False),
        donate_argnums=donate, keep_unused=True)

    sharding = jax.sharding.NamedSharding(mesh, PartitionSpec("core"))
    concat_in = [
        jax.device_put(
            np.concatenate([np.asarray(in_maps[c][name])
                            for c in range(n_cores)], axis=0), sharding)
        for name in in_names
    ]

    def fresh_zeros():
        return [
            jax.device_put(
                np.zeros((n_cores * z.shape[0], *z.shape[1:]), z.dtype),
                sharding)
            for z in zero_outs
        ]

    out_arrs = None
    times = []
    for _ in range(max(1, n_reps)):
        zs = fresh_zeros()
        for z in zs:
            z.block_until_ready()
        t0 = time.perf_counter()
        out_arrs = sharded(*concat_in, *zs)
        for o in out_arrs:
            o.block_until_ready()
        times.append(time.perf_counter() - t0)

    _LAST_RESULTS["wall_times_s"] = times
    _LAST_RESULTS["exec_time_ns"] = int(min(times) * 1e9)
    return [
        {name: np.asarray(out_arrs[i]).reshape(n_cores, *out_avals[i].shape)[c]
         for i, name in enumerate(out_names)}
        for c in range(n_cores)
    ]


# --------------------------------------------------------------------------
# Host entry point
# --------------------------------------------------------------------------

def _run(cfg, h, src, dst, W, Wb, a, ab, use_sim=False, trace=False):
    N, F, H, O, NCORES = cfg["N"], cfg["F"], cfg["H"], cfg["O"], cfg["NCORES"]
    NS, NW = cfg["NS"], cfg["NW"]
    HO = H * O
    TD = HO + H
    AD = HO + 2 * H

    h = np.asarray(h, np.float32)
    src = np.asarray(src).astype(np.int64)
    dst = np.asarray(dst).astype(np.int64)
    W = np.asarray(W, np.float32)
    Wb = np.asarray(Wb, np.float32)
    a = np.asarray(a, np.float32)
    ab = np.asarray(ab, np.float32)

    plan = _plan(cfg, src, dst)
    cfg = dict(cfg, C=plan["C"])

    Wext, bext = _host_weights(cfg, W, Wb, a, ab)

    # per-core inputs
    NSG = NS * NCORES
    h_pad = np.zeros((NSG, F), np.float32)
    h_pad[:N] = h
    iota_np = np.broadcast_to(np.arange(P, dtype=np.float32), (P, P)).astype(BF16)
    in_maps = []
    for c in range(NCORES):
        in_maps.append({
            "hT": np.ascontiguousarray(
                h_pad[NS * c:NS * (c + 1)].T).astype(BF16),
            "wext": Wext.astype(BF16),
            "bext": bext.reshape(1, -1).astype(BF16),
            "ones1": np.ones((1, P), BF16),
            "iota": iota_np,
            "src_idx": plan["src_idx"][c],
            "dst_idx": plan["dst_idx"][c],
            "dstl": plan["dstl"][c],
        })

    nc = build_gat_bass(cfg)
    nc.compile()

    if use_sim:
        from concourse import bass_interp
        sim = bass_interp.MultiCoreSim(nc, NCORES)
        for c in range(NCORES):
            for k, v in in_maps[c].items():
                sim.cores[c].tensor(k)[:] = v
        sim.simulate()
        outs = [np.array(sim.cores[c].mem_tensor("out_local"))
                for c in range(NCORES)]
    else:
        results = _run_pjrt_timed(nc, in_maps, NCORES,
                                  n_reps=3 if trace else 1)
        outs = [results[c]["out_local"] for c in range(NCORES)]

    # unscramble rows + columns
    slot_of, pos_of = plan["slot_of"], plan["pos_of"]
    nodes = np.arange(N)
    rows = slot_of[nodes] * P + pos_of[nodes]
    out = np.empty((N, HO), np.float32)
    for c in range(NCORES):
        lo, hi = NS * c, min(NS * (c + 1), N)
        out[lo:hi] = outs[c][rows[lo:hi]]
    # column map: ref col h*O+o <- ours o*H+h
    hh, oo = np.meshgrid(np.arange(H), np.arange(O), indexing="ij")
    colmap = (oo * H + hh).reshape(-1)
    return out[:, colmap]


def kernel(h, src, dst, W, Wb, a, ab):
    cfg = dict(FULL_CFG)
    cfg["NS"] = 6272
    cfg["NW"] = 50
    trace = os.environ.get("GAT_TRACE", "0") == "1"
    return _run(cfg, h, src, dst, W, Wb, a, ab, use_sim=False, trace=trace)

